# revision 1
# baseline (speedup 1.0000x reference)
"""Chamfer-KL loss kernel for Trainium2 (8 NeuronCores, batch-parallel).

Per core: one batch sample.
  M[i,j] = mu_p[i]@mu_g[j] - 0.5||mu_p[i]||^2 - 0.5||mu_g[j]||^2  (= -dist/2)
computed as a single bf16 matmul with the norm terms folded in as two extra
contraction rows (K=34). Two passes ([i,j] and [j,i] layouts) so both argmax
directions are free-axis scans.

The per-row argmax uses a custom DVE op (one pass, two streams): it pair-folds
the two halves of the row, tracks the running max via an in-body scan, and
emits packed indices 2q+b at new-max positions; accum(MAX) keeps the packed
argmax. The gather tables are interleaved on the host so the packed value is
directly the gather row offset. Indices drive an indirect-DMA gather of the
fp32 (mu, logvar) rows, and the KL is computed exactly in fp32 on-chip.
"""

import numpy as np

BS, N, D = 8, 4096, 32
NT = N // 128  # 32 partition tiles
H = N // 2     # half-row for the fold-by-2 argmax
KAUG = D + 2   # 32 features + norm row + ones row

_NC_CACHE = {}
_OP_CACHE = {}


def _get_argmax_op():
    """Fold-by-2 packed-argmax custom DVE op, registered at runtime."""
    if "op" in _OP_CACHE:
        return _OP_CACHE["op"]
    from concourse.dve_spec import (
        Spec, Src0, Src1, C1, Zero, AluOp, maxx, scan, lower, Bin,
    )
    import concourse.dve_ops as dve_ops
    from concourse.dve_uop import DveOpSpec

    # m_k = max(a_k, b_k); where m_k is a new running max (>= all prefix),
    # emit 2k + [b_k > a_k]; accum(MAX) keeps the last emission.
    idx2 = scan(AluOp.ADD, C1, init=Bin(AluOp.SUBTRACT, Zero, C1))
    c2 = Src1 > Src0
    g = idx2 + c2
    m = maxx(Src0, Src1)
    r = scan(AluOp.MAX, m)
    body = g * (m >= r)

    def ref(in0, in1, c0, c1, imm2):
        a = np.asarray(in0, np.float32)
        b = np.asarray(in1, np.float32)
        m = np.maximum(a, b)
        r = np.maximum.accumulate(m, axis=-1)
        cond = (m >= r).astype(np.float32)
        c1v = c1 if isinstance(c1, float) else np.asarray(c1, np.float32)
        idx2 = np.arange(a.shape[-1], dtype=np.float32) * c1v
        body = (idx2 + (b > a).astype(np.float32)) * cond
        acc = body.max(axis=-1, keepdims=True)
        return body, acc

    spec = Spec(body=body, accum=AluOp.MAX, reference=ref)
    shas = {
        ver: DveOpSpec(
            name="ARGMAX_FOLD2_ANT", opcode=1, uops=lower(spec, ver=ver),
            rd1_en=True,
        ).sha(ver)
        for ver in ("v3", "v4")
    }
    op = dve_ops.DveOp("ARGMAX_FOLD2_ANT", spec, subdim=False, uops_sha=shas)
    if all(o.name != op.name for o in dve_ops.OPS):
        dve_ops.OPS.append(op)
        dve_ops.CUSTOM_DVE_SPECS[op.name] = op.spec
        dve_ops._SUB_OPCODE_FOR_NAME[op.name] = (
            dve_ops._CUSTOM_DVE_ROW_BASE + len(dve_ops.OPS) - 1
        )
    _OP_CACHE["op"] = op
    return op


def _build():
    from contextlib import ExitStack

    import concourse.mybir as mybir
    from concourse import bacc
    from concourse.bass import IndirectOffsetOnAxis
    from concourse.tile import TileContext

    f32 = mybir.dt.float32
    bf16 = mybir.dt.bfloat16
    fp16 = mybir.dt.float16
    u32 = mybir.dt.uint32
    AF = mybir.ActivationFunctionType
    argmax_op = _get_argmax_op()

    nc = bacc.Bacc(None, target_bir_lowering=False)
    xT = nc.dram_tensor("xT", [KAUG, N], bf16, kind="ExternalInput")
    yT = nc.dram_tensor("yT", [KAUG, N], bf16, kind="ExternalInput")
    cat_p = nc.dram_tensor("cat_p", [N, 2 * D], f32, kind="ExternalInput")
    cat_g = nc.dram_tensor("cat_g", [N, 2 * D], f32, kind="ExternalInput")
    # interleaved copies for the fold-argmax gather (row 2q+b = orig q+2048b)
    cat_p2 = nc.dram_tensor("cat_p2", [N, 2 * D], f32, kind="ExternalInput")
    cat_g2 = nc.dram_tensor("cat_g2", [N, 2 * D], f32, kind="ExternalInput")
    # host-pretransposed [p][t][c] copies: loads as 128 contiguous 8KB rows
    # instead of a 4096x256B descriptor storm (~20us of startup DMA)
    natp_h = nc.dram_tensor("natp_h", [128, NT * 2 * D], f32, kind="ExternalInput")
    natg_h = nc.dram_tensor("natg_h", [128, NT * 2 * D], f32, kind="ExternalInput")
    loss = nc.dram_tensor("loss", [1, 1], f32, kind="ExternalOutput")

    with TileContext(nc) as tc:
        with ExitStack() as ctx:
            const = ctx.enter_context(tc.tile_pool(name="const", bufs=1))
            stage_pool = ctx.enter_context(tc.tile_pool(name="stage", bufs=4))
            scr_pool = ctx.enter_context(tc.tile_pool(name="scr", bufs=2))
            # two 4-bank PSUM regions: A holds the scan's in0 half (read
            # directly by the custom DVE op, no staging); B transits the
            # staged half through an ACT fp16 copy.
            psumA = ctx.enter_context(tc.tile_pool(name="psA", bufs=1, space="PSUM"))
            psumB = ctx.enter_context(tc.tile_pool(name="psB", bufs=1, space="PSUM"))
            small = ctx.enter_context(tc.tile_pool(name="small", bufs=4))

            # stationary operands duplicated at base_partition 64: row-group
            # tiling lets two K=34 matmuls run concurrently in PE row groups
            # {0,1} and {2,3} (2x matmul throughput for this thin-K problem)
            xT_sb = const.tile([64 + KAUG, N], bf16, tag="xT_sb")
            yT_sb = const.tile([64 + KAUG, N], bf16, tag="yT_sb")
            nat_p = const.tile([128, NT, 2 * D], f32, tag="nat_p")
            nat_g = const.tile([128, NT, 2 * D], f32, tag="nat_g")
            pargs_y = const.tile([128, NT], f32, tag="pargs_y")
            pargs_x = const.tile([128, NT], f32, tag="pargs_x")
            args_y = const.tile([128, NT], u32, tag="args_y")
            args_x = const.tile([128, NT], u32, tag="args_x")

            # spread the input loads over per-engine DGE queues — serially on
            # one queue they cost ~28us of startup before the first scan
            nc.sync.dma_start(out=xT_sb[0:KAUG, :], in_=xT[:, :])
            nc.scalar.dma_start(out=xT_sb[64 : 64 + KAUG, :], in_=xT[:, :])
            nc.gpsimd.dma_start(out=yT_sb[0:KAUG, :], in_=yT[:, :])
            nc.sync.dma_start(out=yT_sb[64 : 64 + KAUG, :], in_=yT[:, :])
            nc.scalar.dma_start(out=nat_p[:, :, :], in_=natp_h[:, :])
            nc.gpsimd.dma_start(out=nat_g[:, :, :], in_=natg_h[:, :])

            gath_g = const.tile([128, NT, 2 * D], f32, tag="gath_g")
            gath_p = const.tile([128, NT, 2 * D], f32, tag="gath_p")
            # gather landing pads: each per-tile ACT copy depends on exactly
            # one gather (one precise sem wait); the wide KL readers then
            # depend on the ACT engine tick only. Without this, a wide reader
            # of 32 gathers needs waits on all 8 round-robin DMA sems, but
            # tile sem assignment emits only one — a real race once the
            # compute side is fast enough to catch the gathers.
            land_g = const.tile([128, NT, 2 * D], f32, tag="land_g")
            land_p = const.tile([128, NT, 2 * D], f32, tag="land_p")

            # --- main scans: layout A (stationary=x) then layout B ---
            # Software-pipelined: the staged half of tile t+1 is emitted
            # BEFORE the scan half of tile t, so the in-order PE queue can
            # fill psumB while the scan still holds psumA (no head-of-line
            # blocking behind the psumA hold).
            for stat_sb, mov_sb, pargs, args, table, gath, land in (
                (xT_sb, yT_sb, pargs_y, args_y, cat_g2, gath_g, land_g),
                (yT_sb, xT_sb, pargs_x, args_x, cat_p2, gath_p, land_p),
            ):
                def fill_staged(t):
                    stat_lo = stat_sb[0:KAUG, t * 128 : (t + 1) * 128]
                    stat_hi = stat_sb[64 : 64 + KAUG, t * 128 : (t + 1) * 128]
                    psB = psumB.tile([128, H], f32, tag="psB")
                    for q in (0, 2):
                        i1 = nc.tensor.matmul(
                            psB[:, q * 512 : (q + 1) * 512],
                            lhsT=stat_lo,
                            rhs=mov_sb[0:KAUG, (4 + q) * 512 : (5 + q) * 512],
                            start=True,
                            stop=True,
                        )
                        i2 = nc.tensor.matmul(
                            psB[:, (q + 1) * 512 : (q + 2) * 512],
                            lhsT=stat_hi,
                            rhs=mov_sb[64 : 64 + KAUG, (5 + q) * 512 : (6 + q) * 512],
                            start=True,
                            stop=True,
                        )
                        if q == 2:
                            # same stationary tile as the q=0 pair: reuse the
                            # weights already resident in the PE array
                            i1.ins.ldweights = False
                            i2.ins.ldweights = False
                    sB = stage_pool.tile([128, H], fp16, tag="stageB")
                    nc.scalar.copy(out=sB[:, :], in_=psB[:, :])
                    return sB

                stageB = fill_staged(0)
                for t in range(NT):
                    next_stageB = fill_staged(t + 1) if t + 1 < NT else None
                    stat_lo = stat_sb[0:KAUG, t * 128 : (t + 1) * 128]
                    stat_hi = stat_sb[64 : 64 + KAUG, t * 128 : (t + 1) * 128]
                    # scanned half (cols 0..2047) stays in PSUM
                    psA = psumA.tile([128, H], f32, tag="psA")
                    for q in (0, 2):
                        i1 = nc.tensor.matmul(
                            psA[:, q * 512 : (q + 1) * 512],
                            lhsT=stat_lo,
                            rhs=mov_sb[0:KAUG, q * 512 : (q + 1) * 512],
                            start=True,
                            stop=True,
                        )
                        i2 = nc.tensor.matmul(
                            psA[:, (q + 1) * 512 : (q + 2) * 512],
                            lhsT=stat_hi,
                            rhs=mov_sb[64 : 64 + KAUG, (q + 1) * 512 : (q + 2) * 512],
                            start=True,
                            stop=True,
                        )
                        if q == 2:
                            i1.ins.ldweights = False
                            i2.ins.ldweights = False
                    # one-pass packed argmax (accum must be f32: the ISA
                    # rejects int accumulator dtypes) + tiny u32 cast
                    scr = scr_pool.tile([128, H], fp16, tag="scr")
                    nc.vector._custom_dve(
                        argmax_op,
                        out=scr[:, :],
                        in0=psA[:, :],
                        in1=stageB[:, :],
                        s1=2.0,
                        accum_out=pargs[:, t : t + 1],
                    )
                    nc.vector.tensor_scalar_add(
                        args[:, t : t + 1], pargs[:, t : t + 1], 0.0
                    )
                    # gather (mu|logvar) row from the INTERLEAVED table
                    nc.gpsimd.indirect_dma_start(
                        gath[:, t, :],
                        None,
                        table[:, :],
                        IndirectOffsetOnAxis(ap=args[:, t : t + 1], axis=0),
                    )
                    nc.vector.tensor_scalar_add(
                        land[:, t, :], gath[:, t, :], 0.0
                    )
                    stageB = next_stageB

            # --- exact fp32 KL on gathered rows ---
            # Split over NT halves: the chunks whose gathers land early can
            # run under the remaining scan work instead of in the tail.
            klacc = const.tile([128, NT], f32, tag="klacc")
            HT = NT // 2

            def kl_side(pv, ov, first, sfx, lo, hi):
                # S = sum_d (t1 - exp(t1) - (mu_p-mu_o)^2 * exp(-lv_o)),
                # with t1 = lv_p - lv_o.  (the "+1" per dim is folded in later)
                nt = hi - lo
                mu_pv = pv[:, lo:hi, 0:D]
                lv_pv = pv[:, lo:hi, D : 2 * D]
                mu_ov = ov[:, lo:hi, 0:D]
                lv_ov = ov[:, lo:hi, D : 2 * D]
                sc1 = const.tile([128, nt, D], f32, tag="sc1" + sfx)
                sc2 = const.tile([128, nt, D], f32, tag="sc2" + sfx)
                sc3 = const.tile([128, nt, D], f32, tag="sc3" + sfx)
                nc.vector.tensor_sub(sc1[:, :, :], lv_pv, lv_ov)
                nc.scalar.activation(sc2[:, :, :], sc1[:, :, :], AF.Exp)
                nc.vector.tensor_sub(sc1[:, :, :], sc1[:, :, :], sc2[:, :, :])
                nc.vector.tensor_sub(sc2[:, :, :], mu_pv, mu_ov)
                nc.scalar.activation(sc2[:, :, :], sc2[:, :, :], AF.Square)
                nc.scalar.activation(sc3[:, :, :], lv_ov, AF.Exp, scale=-1.0)
                nc.vector.tensor_mul(sc2[:, :, :], sc2[:, :, :], sc3[:, :, :])
                nc.vector.tensor_sub(sc1[:, :, :], sc1[:, :, :], sc2[:, :, :])
                if first:
                    nc.vector.reduce_sum(
                        klacc[:, lo:hi], sc1[:, :, :], axis=mybir.AxisListType.X
                    )
                else:
                    red = small.tile([128, nt], f32, tag="red" + sfx)
                    nc.vector.reduce_sum(
                        red[:, :], sc1[:, :, :], axis=mybir.AxisListType.X
                    )
                    nc.vector.tensor_add(
                        klacc[:, lo:hi], klacc[:, lo:hi], red[:, :]
                    )

            # loss_2 side: p = natural preds, o = gathered gts
            kl_side(nat_p, land_g, first=True, sfx="a", lo=0, hi=NT)
            # loss_1 side: p = gathered preds, o = natural gts
            kl_side(land_p, nat_g, first=False, sfx="b", lo=0, hi=NT)
            # fold the two "+ sum_d 1 = +D" constants (one per side)
            nc.vector.tensor_scalar_add(klacc[:, :], klacc[:, :], float(2 * D))

            # partition-sum via ones-vector matmul (exact fp32 in PSUM)
            ones_col = const.tile([128, 1], f32, tag="ones_col")
            nc.vector.memset(ones_col[:, :], 1.0)
            ps_fin = psumA.tile([128, H], f32, tag="psA")
            nc.tensor.matmul(
                ps_fin[0:1, 0:NT],
                lhsT=ones_col[:, :],
                rhs=klacc[:, :],
                start=True,
                stop=True,
            )
            fin = small.tile([1, 1], f32, tag="fin")
            nc.vector.reduce_sum(
                fin[:, :], ps_fin[0:1, 0:NT], axis=mybir.AxisListType.X
            )
            # loss = 0.5*(l1+l2), each l = -0.5*S  ->  -0.25*(S1+S2)
            nc.vector.tensor_scalar_mul(fin[:, :], fin[:, :], -0.25)
            nc.sync.dma_start(out=loss[:, :], in_=fin[:, :])

    nc.finalize()
    return nc


def _get_nc():
    if "nc" not in _NC_CACHE:
        _NC_CACHE["nc"] = _build()
    return _NC_CACHE["nc"]


def _host_prep(mu_p, lv_p, mu_g, lv_g):
    """Per-sample input marshalling: bf16 transposed/augmented matmul
    operands and the fp32 (mu|logvar) gather tables.

    The gather tables are interleaved so that the packed fold-argmax value
    k = 2q + b maps directly to the original row q + 2048*b."""
    import ml_dtypes

    bf16 = ml_dtypes.bfloat16
    x = mu_p.astype(bf16)
    y = mu_g.astype(bf16)
    xf = x.astype(np.float32)
    yf = y.astype(np.float32)
    ax = (-0.5 * np.sum(xf * xf, -1)).astype(bf16)
    ay = (-0.5 * np.sum(yf * yf, -1)).astype(bf16)
    ones = np.ones((N,), bf16)
    xT = np.ascontiguousarray(np.concatenate([x.T, ax[None, :], ones[None, :]], 0))
    yT = np.ascontiguousarray(np.concatenate([y.T, ones[None, :], ay[None, :]], 0))
    k = np.arange(N)
    perm = (k // 2) + H * (k % 2)
    cat_p = np.ascontiguousarray(np.concatenate([mu_p, lv_p], 1).astype(np.float32))
    cat_g = np.ascontiguousarray(np.concatenate([mu_g, lv_g], 1).astype(np.float32))
    natp_h = np.ascontiguousarray(
        cat_p.reshape(NT, 128, 2 * D).transpose(1, 0, 2).reshape(128, -1)
    )
    natg_h = np.ascontiguousarray(
        cat_g.reshape(NT, 128, 2 * D).transpose(1, 0, 2).reshape(128, -1)
    )
    return {
        "xT": xT,
        "yT": yT,
        "cat_p": cat_p,
        "cat_g": cat_g,
        "cat_p2": np.ascontiguousarray(cat_p[perm]),
        "cat_g2": np.ascontiguousarray(cat_g[perm]),
        "natp_h": natp_h,
        "natg_h": natg_h,
    }


def make_in_maps(mu_preds, logvar_preds, mu_gts, logvar_gts):
    mu_preds = np.asarray(mu_preds, dtype=np.float32)
    logvar_preds = np.asarray(logvar_preds, dtype=np.float32)
    mu_gts = np.asarray(mu_gts, dtype=np.float32)
    logvar_gts = np.asarray(logvar_gts, dtype=np.float32)
    return [
        _host_prep(mu_preds[b], logvar_preds[b], mu_gts[b], logvar_gts[b])
        for b in range(BS)
    ]


def run(in_maps, trace=False):
    from concourse.bass_utils import run_bass_kernel_spmd

    nc = _get_nc()
    res = run_bass_kernel_spmd(nc, in_maps, list(range(BS)), trace=trace)
    out = np.array(
        [np.asarray(res.results[b]["loss"]).reshape(()) for b in range(BS)],
        dtype=np.float32,
    )
    return out, res


def kernel(mu_preds, logvar_preds, mu_gts, logvar_gts):
    in_maps = make_in_maps(mu_preds, logvar_preds, mu_gts, logvar_gts)
    out, _ = run(in_maps)
    return out

